# revision 26
# baseline (speedup 1.0000x reference)
"""CapsNet-CIFAR100 forward on 8 Trainium2 NeuronCores.

Hybrid sharding: conv stem data-parallel (8 images/core), dynamic routing
out-capsule-sharded (each core: all 64 images x 13 o-slots, W shard
SBUF/stream-resident).  u_hat (26M elems) is never materialized:

  pass 0:  s0 = 0.01 * sum_ik W u           (PE, (i,k)-contraction)
  dm path: y = sum_d W v  (PE, half of u_hat), dm = sum_k y*u (DVE folds)
  softmax over o is GLOBAL: z partial-summed per core, AllReduce'd.
  s path:  cu = c*u (DVE), s = sum_ik W cu  (PE, (i,k)-contraction)

Index maps (per core):  i = co*8+oh, k = ow;  b in [64];  o in [13] slots.
  P1 = oh*16 + b%16 (quads q=b//16)   "dm layout"  [p, (o, q, co)]
  P2 = co%128 (halves h=co//128)      "s layout"   [p, (h, oh, ow, ...)]
Collectives: AllGather of u (262KB/core) after conv; AllReduce of
z=[128,1024] f16 per routing pass.
"""

from contextlib import ExitStack

import numpy as np
import concourse.bass as bass
import concourse.mybir as mybir
import concourse.tile as tile
from concourse import bacc
from concourse import bass_utils

F32 = mybir.dt.float32
F16 = mybir.dt.float16
AF = mybir.ActivationFunctionType
ALU = mybir.AluOpType
AX = mybir.AxisListType

N_CORES = 8
B = 8            # conv batch per core
NO = 13          # o-slots per core (padded)
SIZES = [13, 13, 13, 13, 12, 12, 12, 12]
OFFS = [0, 13, 26, 39, 52, 64, 76, 88]
EPS = 1e-8

_CACHE = {}


def _build():
    nc = bacc.Bacc("TRN2", target_bir_lowering=False, debug=False,
                   num_devices=N_CORES)
    RG = [list(range(N_CORES))]

    # conv inputs (as baseline)
    xd = nc.dram_tensor("x_sh", [B, 3, 32, 32], F32, kind="ExternalInput").ap()
    w1d = nc.dram_tensor("w1t", [9, 27, 256], F16, kind="ExternalInput").ap()
    cbd = nc.dram_tensor("cb", [256, 1], F32, kind="ExternalInput").ap()
    w2d = nc.dram_tensor("w2t", [2, 128, 81, 256], F16, kind="ExternalInput").ap()
    pbd = nc.dram_tensor("pb", [1, 256], F32, kind="ExternalInput").ap()
    # routing inputs
    wyd = nc.dram_tensor("wy", [NO, 128, 2048], F16, kind="ExternalInput").ap()
    wsd = nc.dram_tensor("ws", [NO, 128, 2048], F16, kind="ExternalInput").ap()
    ws0d = nc.dram_tensor("ws0", [8, 128, 16 * NO * 16], F16, kind="ExternalInput").ap()
    rsd = nc.dram_tensor("repsel", [16, 128], F16, kind="ExternalInput").ap()
    obd = nc.dram_tensor("obias", [128, NO], F16, kind="ExternalInput").ap()
    bmd = nc.dram_tensor("blkmask", [128, 128], F16, kind="ExternalInput").ap()
    idd = nc.dram_tensor("ident", [128, 128], F16, kind="ExternalInput").ap()
    # scratch / comm
    fdram = nc.dram_tensor("fscratch", [4, 16, 256], F32, kind="Internal").ap()
    ubd_t = nc.dram_tensor("ub_d", [B * 16384], F16, kind="Internal").ap()
    uall_t = nc.dram_tensor("uall_d", [64 * 16384], F16, kind="Internal",
                            addr_space="Shared").ap()
    zbd = [nc.dram_tensor(f"zb_d{i}", [128 * 512], F16, kind="Internal").ap()
           for i in range(2)]
    zrd = [nc.dram_tensor(f"zr_d{i}", [128 * 512], F16, kind="Internal",
                          addr_space="Shared").ap() for i in range(2)]
    vout = nc.dram_tensor("v_out", [64, NO, 16], F32, kind="ExternalOutput").ap()

    with tile.TileContext(nc) as tc:
        with ExitStack() as stack:
            cpool = stack.enter_context(tc.tile_pool(name="consts", bufs=1))

            # ---------- shared constants ----------
            w1sb = cpool.tile([27, 9, 256], F16, name="w1sb")
            nc.sync.dma_start(out=w1sb, in_=w1d.rearrange("k c o -> c k o"))
            cbsb = cpool.tile([128, 2, 1], F32, name="cbsb")
            nc.sync.dma_start(out=cbsb, in_=cbd.rearrange("(t p) one -> p t one", p=128))
            pbrep = cpool.tile([128, 256], F32, name="pbrep")
            nc.sync.dma_start(
                out=pbrep,
                in_=bass.AP(tensor=pbd.tensor, offset=0, ap=[[0, 128], [1, 256]]))
            epssb = cpool.tile([128, 1], F32, name="epssb")
            nc.vector.memset(epssb, EPS)
            gsb = cpool.tile([128, 16], F16, name="gsb")
            # g[p, p//8] = 1 selector for squash-u (built on host side of blkmask? no: iota)
            # build via memset+iota is awkward; reuse host input blkmask? separate input:
            # we fold it into obias? -> simplest: device-side from ident via DMA is messy.
            # Use dedicated host input below (gmat).
            bmsb = cpool.tile([128, 128], F16, name="bmsb")
            nc.sync.dma_start(out=bmsb, in_=bmd)
            idsb = cpool.tile([128, 128], F16, name="idsb")
            nc.sync.dma_start(out=idsb, in_=idd)
            obsb = cpool.tile([128, NO], F16, name="obsb")
            nc.sync.dma_start(out=obsb, in_=obd)
            rssb = cpool.tile([16, 128], F16, name="rssb")
            nc.sync.dma_start(out=rssb, in_=rsd)

            gmd = nc.dram_tensor("gmat", [128, 16], F16, kind="ExternalInput").ap()
            nc.sync.dma_start(out=gsb, in_=gmd)

            # ---------- conv stages (baseline, stages A-C) ----------
            with tc.tile_pool(name="work", bufs=2) as wpool, \
                 tc.tile_pool(name="acts", bufs=1) as apool:
                with tc.tile_pool(name="hpool", bufs=1) as hpool:
                    hsb = [hpool.tile([128, B, 24, 24], F16, name="hsb",
                                      tag=f"h{c}") for c in range(2)]
                    with tc.tile_pool(name="imp", bufs=1) as impool, \
                         tc.tile_pool(name="psc", bufs=2, space="PSUM") as pscpool:
                        xsf = impool.tile([27, B, 32, 24], F32, name="xsf")
                        for ci in range(3):
                            for kw in range(9):
                                src = bass.AP(
                                    tensor=xd.tensor,
                                    offset=ci * 1024 + kw,
                                    ap=[[3072, B], [32, 32], [1, 24]],
                                )
                                nc.sync.dma_start(
                                    out=xsf[ci * 9 + kw:ci * 9 + kw + 1], in_=src)
                        xsb = impool.tile([27, B, 32, 24], F16, name="xsb")
                        nc.vector.tensor_copy(xsb, xsf)

                        for oc in range(2):
                            for b in range(B):
                                for hh in range(2):
                                    ph = pscpool.tile([128, 288], F32, name="ph",
                                                      tag="pconv")
                                    for kh in range(9):
                                        nc.tensor.matmul(
                                            ph,
                                            lhsT=w1sb[:, kh, oc * 128:(oc + 1) * 128],
                                            rhs=xsb[:, b, kh + hh * 12:
                                                    kh + hh * 12 + 12, :].rearrange(
                                                        "c h w -> c (h w)"),
                                            start=(kh == 0), stop=(kh == 8),
                                        )
                                    nc.scalar.activation(
                                        hsb[oc][:, b, hh * 12:(hh + 1) * 12, :].rearrange(
                                            "p h w -> p (h w)"),
                                        ph, AF.Relu, bias=cbsb[:, oc],
                                    )

                    # stage B: conv2 (transposed) -> p2sb
                    p2sb = [apool.tile([128, 256], F32, name="p2sb",
                                       tag=f"p2sb{bp}") for bp in range(4)]
                    with tc.tile_pool(name="w2", bufs=2) as w2pool, \
                         tc.tile_pool(name="psc2", bufs=1, space="PSUM") as psc2pool:
                        p2ps = [psc2pool.tile([128, 256], F32, name="p2ps",
                                              tag=f"p2ps{bp}") for bp in range(4)]
                        nmm = [0, 0, 0, 0]
                        for g in range(9):
                            w2g = [w2pool.tile([128, 9, 256], F16, name="w2g",
                                               tag="w2g") for _ in range(2)]
                            for cic in range(2):
                                nc.sync.dma_start(out=w2g[cic],
                                                  in_=w2d[cic, :, g * 9:(g + 1) * 9, :])
                            for j in range(9):
                                khw = g * 9 + j
                                kh, kw = khw // 9, khw % 9
                                for cic in range(2):
                                    hshift = wpool.tile([128, B, 8, 8], F16,
                                                        name="hshift", tag="hshift")
                                    if cic == 0:
                                        nc.vector.tensor_copy(
                                            hshift,
                                            hsb[cic][:, :, kh:kh + 16:2, kw:kw + 16:2])
                                    else:
                                        nc.scalar.copy(
                                            hshift,
                                            hsb[cic][:, :, kh:kh + 16:2, kw:kw + 16:2])
                                    hflat = hshift.rearrange("p b h w -> p (b h w)")
                                    for bp in range(4):
                                        nc.tensor.matmul(
                                            p2ps[bp],
                                            lhsT=hflat[:, bp * 128:(bp + 1) * 128],
                                            rhs=w2g[cic][:, j, :],
                                            start=(nmm[bp] == 0), stop=(nmm[bp] == 161),
                                        )
                                        nmm[bp] += 1
                        for bp in range(4):
                            nc.vector.tensor_tensor(out=p2sb[bp], in0=p2ps[bp],
                                                    in1=pbrep, op=ALU.add)

                # stage C: squash over ow -> ub (bf16)
                ub = [apool.tile([128, 256], F16, name="ub", tag=f"ub{bp}")
                      for bp in range(4)]
                with tc.tile_pool(name="psn", bufs=2, space="PSUM") as psnpool:
                    for bp in range(4):
                        sq = wpool.tile([128, 256], F16, name="sq", tag="sq")
                        nc.vector.tensor_mul(sq, p2sb[bp], p2sb[bp])
                        n2ps = psnpool.tile([16, 256], F32, name="n2ps", tag="n2ps")
                        nc.tensor.matmul(n2ps, lhsT=gsb,
                                         rhs=sq, start=True, stop=True)
                        n2 = wpool.tile([16, 256], F32, name="n2", tag="n2")
                        nc.scalar.activation(n2, n2ps, AF.Copy)
                        r1 = wpool.tile([16, 256], F32, name="r1", tag="r1")
                        nc.vector.tensor_scalar_add(r1, in0=n2, scalar1=1.0)
                        nc.vector.reciprocal(r1, r1)
                        q = wpool.tile([16, 256], F32, name="q", tag="q")
                        nc.scalar.activation(q, n2, AF.Sqrt, bias=epssb[:16])
                        nc.vector.reciprocal(q, q)
                        f = wpool.tile([16, 256], F32, name="f", tag="f")
                        nc.vector.tensor_mul(f, n2, r1)
                        nc.vector.tensor_mul(f, f, q)
                        nc.sync.dma_start(out=fdram[bp], in_=f)
                        frep = wpool.tile([128, 256], F32, name="frep", tag="frep")
                        for grp in range(16):
                            nc.sync.dma_start(
                                out=frep[grp * 8:(grp + 1) * 8, :],
                                in_=bass.AP(tensor=fdram.tensor,
                                            offset=(bp * 16 + grp) * 256,
                                            ap=[[0, 8], [1, 256]]))
                        nc.vector.tensor_tensor(out=ub[bp], in0=p2sb[bp], in1=frep,
                                                op=ALU.mult)

                # export u to DRAM: ub_d[b*16384 + oh*2048 + ow*256 + co]
                # (co contiguous so every DMA run is 512B)
                for bp in range(4):
                    for bl in range(2):
                        dst = bass.AP(tensor=ubd_t.tensor,
                                      offset=(bp * 2 + bl) * 16384,
                                      ap=[[256, 64], [1, 256]])
                        nc.sync.dma_start(out=dst,
                                          in_=ub[bp][bl * 64:(bl + 1) * 64, :])

            # ---------- u AllGather ----------
            nc.gpsimd.collective_compute(
                "AllGather", ALU.bypass, replica_groups=RG,
                ins=[ubd_t.opt()], outs=[uall_t.opt()])

            # ---------- routing-persistent tiles + u relayouts ----------
            rpool = stack.enter_context(tc.tile_pool(name="rp", bufs=1))
            u_y = rpool.tile([128, 4, 256, 8], F16, name="u_y")
            u_s = rpool.tile([128, 2, 8, 8, 64], F16, name="u_s")
            with tc.tile_pool(name="rly", bufs=1) as rlpool, \
                 tc.tile_pool(name="rlps", bufs=4, space="PSUM") as rlps:
                ug = rlpool.tile([64, 16384], F16, name="ug")
                nc.sync.dma_start(
                    out=ug,
                    in_=bass.AP(tensor=uall_t.tensor, offset=0,
                                ap=[[16384, 64], [1, 16384]]))
                # T1: u_s[p=co%128, (h,oh,ow,b)] via per-(h,oh,ow) transposes
                ugv = ug.rearrange("b (g w c) -> b g w c", g=8, w=8)
                for h in range(2):
                    for oh in range(8):
                        for ow in range(8):
                            t1 = rlps.tile([128, 64], F16, name="t1", tag="t1")
                            nc.tensor.transpose(
                                t1, ugv[:, oh, ow, h * 128:(h + 1) * 128],
                                idsb[:64, :64])
                            nc.scalar.activation(u_s[:, h, oh, ow, :], t1, AF.Copy)
                # u_mid[co, (h, k, q, oh, b16)] = u_s[co, h, oh, k, q*16+b16]
                umid = rlpool.tile([128, 2, 8, 4, 8, 16], F16, name="umid")
                nc.vector.tensor_copy(
                    umid,
                    u_s.rearrange("p h oh k (q s) -> p h k q oh s", q=4))
                # T2: u_y[p=(oh,b16), (q, co, k)]
                for h in range(2):
                    for k in range(8):
                        for q in range(4):
                            t2 = rlps.tile([128, 128], F16, name="t2", tag="t2")
                            nc.tensor.transpose(t2, umid[:, h, k, q].rearrange(
                                "p a s -> p (a s)"), idsb)
                            nc.scalar.activation(
                                u_y[:, q, h * 128:(h + 1) * 128, k],
                                t2, AF.Copy)

            dm0 = rpool.tile([128, NO, 4, 256], F16, name="dm0")
            dm1 = rpool.tile([128, NO, 4, 256], F16, name="dm1")
            vrep = rpool.tile([128, NO, 64], F16, name="vrep")
            s_all = rpool.tile([64, NO, 16], F32, name="s_all")
            zsum = rpool.tile([128, 1024], F16, name="zsum")
            zf32 = rpool.tile([128, 1024], F32, name="zf32")
            zi16 = rpool.tile([128, 4, 256], F16, name="zi16")
            zp = rpool.tile([128, 4, 256], F16, name="zp")

            vpool = stack.enter_context(tc.tile_pool(name="vp", bufs=1))
            vpsp = stack.enter_context(tc.tile_pool(name="vpsp", bufs=1, space="PSUM"))
            vpsp2 = stack.enter_context(tc.tile_pool(name="vpsp2", bufs=1, space="PSUM"))

            def squash(t):
                """s_all [64, NO, 16] f32 -> v; t=2 writes vout, else vrep."""
                sq = vpool.tile([64, NO, 16], F32, name="ssq", tag="ssq")
                nc.vector.tensor_mul(sq, s_all, s_all)
                n2 = vpool.tile([64, NO], F32, name="sn2", tag="sn2")
                nc.vector.tensor_reduce(n2, sq, axis=AX.X, op=ALU.add)
                r1 = vpool.tile([64, NO], F32, name="sr1", tag="sr1")
                nc.vector.tensor_scalar_add(r1, in0=n2, scalar1=1.0)
                nc.vector.reciprocal(r1, r1)
                qq = vpool.tile([64, NO], F32, name="sqq", tag="sqq")
                nc.scalar.activation(qq, n2, AF.Sqrt, bias=epssb[:64])
                nc.vector.reciprocal(qq, qq)
                ff = vpool.tile([64, NO], F32, name="sff", tag="sff")
                nc.vector.tensor_mul(ff, n2, r1)
                nc.vector.tensor_mul(ff, ff, qq)
                vv = vpool.tile([64, NO, 16], F32, name="svv", tag="svv")
                nc.vector.tensor_tensor(
                    out=vv, in0=s_all,
                    in1=ff.unsqueeze(2).broadcast_to([64, NO, 16]), op=ALU.mult)
                if t == 2:
                    nc.sync.dma_start(out=vout, in_=vv)
                else:
                    # vrep[p=(oh,d), (o,b)] via transpose + replication matmul
                    vb16 = vpool.tile([64, NO, 16], F16, name="svb", tag="svb")
                    nc.vector.tensor_copy(vb16, vv)
                    for o in range(NO):
                        tp = vpsp.tile([16, 64], F16, name="vtp", tag="vtp")
                        nc.tensor.transpose(tp, vb16[:, o, :], idsb[:64, :64])
                        vts = vpool.tile([16, 64], F16, name="vts", tag="vts")
                        nc.scalar.activation(vts, tp, AF.Copy)
                        rp = vpsp2.tile([128, 64], F32, name="vrp", tag="vrp")
                        nc.tensor.matmul(rp, lhsT=rssb, rhs=vts,
                                         start=True, stop=True)
                        nc.scalar.activation(vrep[:, o, :], rp, AF.Copy)

            # ---------- pass 0 ----------
            with tc.tile_pool(name="p0", bufs=2) as p0pool, \
                 tc.tile_pool(name="p0c", bufs=1) as p0cpool, \
                 tc.tile_pool(name="p0ps", bufs=1, space="PSUM") as p0psp:
                cu0 = p0cpool.tile([128, 2, 8, 8, 64], F16, name="cu0")
                nc.vector.tensor_scalar_mul(cu0, in0=u_s, scalar1=0.01)
                s0ps = p0psp.tile([64, NO * 16], F32, name="s0ps")
                for g in range(8):
                    w0t = p0pool.tile([128, 16, NO * 16], F16, name="w0t", tag="w0t")
                    nc.sync.dma_start(
                        out=w0t,
                        in_=bass.AP(tensor=ws0d.tensor, offset=g * 128 * 16 * NO * 16,
                                    ap=[[16 * NO * 16, 128], [1, 16 * NO * 16]]))
                    for j in range(16):
                        ch = g * 16 + j
                        h, oh, ow = ch // 64, (ch // 8) % 8, ch % 8
                        nc.tensor.matmul(
                            s0ps, lhsT=cu0[:, h, oh, ow, :], rhs=w0t[:, j, :],
                            start=(ch == 0), stop=(ch == 127))
                nc.scalar.activation(s_all.rearrange("b o d -> b (o d)"), s0ps,
                                     AF.Copy)
                squash(0)

            # ---------- passes 1, 2 ----------
            wypool = stack.enter_context(tc.tile_pool(name="wyp", bufs=2))
            wspool = stack.enter_context(tc.tile_pool(name="wsp", bufs=2))
            ypool = stack.enter_context(tc.tile_pool(name="yp", bufs=3))
            cupool = stack.enter_context(tc.tile_pool(name="cup", bufs=3))
            ctpool = stack.enter_context(tc.tile_pool(name="ctp", bufs=2))
            vbpool = stack.enter_context(tc.tile_pool(name="vbp", bufs=2))
            ypsp = stack.enter_context(tc.tile_pool(name="ypsp", bufs=2, space="PSUM"))
            tpsp = stack.enter_context(tc.tile_pool(name="tpsp", bufs=2, space="PSUM"))
            spsp = stack.enter_context(tc.tile_pool(name="spsp", bufs=2, space="PSUM"))

            for t in (1, 2):
                dmt = dm0 if t == 1 else dm1

                def dm_half(hf):
                    qs = slice(2 * hf, 2 * hf + 2)
                    for o in range(NO):
                        wyt = wypool.tile([128, 2048], F16, name="wyt", tag="wyt")
                        nc.sync.dma_start(
                            out=wyt,
                            in_=bass.AP(tensor=wyd.tensor, offset=o * 128 * 2048,
                                        ap=[[2048, 128], [1, 2048]]))
                        vblk = vbpool.tile([128, 2, 8, 16], F16, name="vblk",
                                           tag="vblk")
                        nc.vector.tensor_tensor(
                            out=vblk,
                            in0=vrep[:, o, 32 * hf:32 * (hf + 1)]
                                .rearrange("p (q s) -> p q s", q=2)
                                .unsqueeze(2).broadcast_to([128, 2, 8, 16]),
                            in1=bmsb.rearrange("p (a s) -> p a s", a=8)
                                .unsqueeze(1).broadcast_to([128, 2, 8, 16]),
                            op=ALU.mult)
                        y16 = ypool.tile([128, 2, 256, 8], F16, name="y16",
                                         tag="y16")
                        for qh in range(2):
                            for cc in range(4):
                                yps = ypsp.tile([128, 512], F32, name="yps",
                                                tag="yps")
                                nc.tensor.matmul(
                                    yps,
                                    lhsT=vblk[:, qh].rearrange("p a s -> p (a s)"),
                                    rhs=wyt[:, cc * 512:(cc + 1) * 512],
                                    start=True, stop=True)
                                nc.scalar.activation(
                                    y16[:, qh, cc * 64:(cc + 1) * 64, :].rearrange(
                                        "p c k -> p (c k)"),
                                    yps, AF.Copy)
                        nc.vector.tensor_tensor(out=y16, in0=y16,
                                                in1=u_y[:, qs], op=ALU.mult)
                        eng = nc.gpsimd if o % 2 == 0 else nc.vector
                        eng.tensor_tensor(out=y16[:, :, :, 0:4],
                                          in0=y16[:, :, :, 0:4],
                                          in1=y16[:, :, :, 4:8], op=ALU.add)
                        nc.vector.tensor_tensor(out=y16[:, :, :, 0:2],
                                                in0=y16[:, :, :, 0:2],
                                                in1=y16[:, :, :, 2:4], op=ALU.add)
                        nc.vector.tensor_tensor(out=dmt[:, o, qs],
                                                in0=y16[:, :, :, 0],
                                                in1=y16[:, :, :, 1], op=ALU.add)
                        if o == NO - 1:
                            nc.gpsimd.tensor_tensor(
                                out=dmt[:, NO - 1, qs], in0=dmt[:, NO - 1, qs],
                                in1=obsb[:, NO - 1:NO].unsqueeze(2)
                                    .broadcast_to([128, 2, 256]),
                                op=ALU.add)

                def softmax_half(hf):
                    qs = slice(2 * hf, 2 * hf + 2)
                    if t == 2:
                        nc.vector.tensor_tensor(out=dm0[:, :, qs],
                                                in0=dm0[:, :, qs],
                                                in1=dm1[:, :, qs], op=ALU.add)
                    nc.scalar.activation(dm1[:, :, qs], dm0[:, :, qs], AF.Exp)
                    zph = zp[:, qs]
                    nc.vector.tensor_tensor(out=zph, in0=dm1[:, 0, qs],
                                            in1=dm1[:, 1, qs], op=ALU.add)
                    for o in range(2, NO):
                        nc.vector.tensor_tensor(out=zph, in0=zph,
                                                in1=dm1[:, o, qs], op=ALU.add)
                    nc.sync.dma_start(
                        out=bass.AP(tensor=zbd[hf].tensor, offset=0,
                                    ap=[[512, 128], [1, 512]]),
                        in_=zph)
                    nc.gpsimd.collective_compute(
                        "AllReduce", ALU.add, replica_groups=RG,
                        ins=[zbd[hf].opt()], outs=[zrd[hf].opt()])
                    nc.sync.dma_start(
                        out=zsum[:, hf * 512:(hf + 1) * 512],
                        in_=bass.AP(tensor=zrd[hf].tensor, offset=0,
                                    ap=[[512, 128], [1, 512]]))
                    zfh = zf32[:, hf * 512:(hf + 1) * 512]
                    nc.vector.tensor_copy(zfh, zsum[:, hf * 512:(hf + 1) * 512])
                    nc.vector.reciprocal(zfh, zfh)
                    nc.vector.tensor_copy(zi16[:, qs], zfh.rearrange(
                        "p (q c) -> p q c", q=2))
                    nc.vector.tensor_tensor(
                        out=dm1[:, :, qs], in0=dm1[:, :, qs],
                        in1=zi16[:, qs].unsqueeze(1)
                            .broadcast_to([128, NO, 2, 256]),
                        op=ALU.mult)

                def s_half(hf):
                    for o in range(NO):
                        wst = wspool.tile([128, 128, 16], F16, name="wst",
                                          tag="wst")
                        nc.sync.dma_start(
                            out=wst,
                            in_=bass.AP(tensor=wsd.tensor, offset=o * 128 * 2048,
                                        ap=[[2048, 128], [1, 2048]]))
                        ct = ctpool.tile([128, 2, 8, 32], F16, name="ct", tag="ct")
                        for qi in range(2):
                            qq_ = 2 * hf + qi
                            for h in range(2):
                                tps = tpsp.tile([128, 128], F16, name="tps",
                                                tag="tps")
                                nc.tensor.transpose(
                                    tps, dm1[:, o, qq_, h * 128:(h + 1) * 128],
                                    idsb)
                                nc.vector.tensor_copy(
                                    ct[:, h, :, qi * 16:(qi + 1) * 16],
                                    tps.rearrange("p (a s) -> p a s", a=8))
                        cu = cupool.tile([128, 2, 8, 8, 32], F16, name="cu",
                                         tag="cu")
                        nc.vector.tensor_tensor(
                            out=cu, in0=u_s[:, :, :, :, 32 * hf:32 * (hf + 1)],
                            in1=ct.unsqueeze(3).broadcast_to([128, 2, 8, 8, 32]),
                            op=ALU.mult)
                        sps = spsp.tile([32, 16], F32, name="sps", tag="sps")
                        for ch in range(128):
                            h, oh, ow = ch // 64, (ch // 8) % 8, ch % 8
                            nc.tensor.matmul(
                                sps, lhsT=cu[:, h, oh, ow, :],
                                rhs=wst[:, ch, :],
                                start=(ch == 0), stop=(ch == 127))
                        nc.scalar.activation(s_all[32 * hf:32 * (hf + 1), o, :],
                                             sps, AF.Copy)

                dm_half(0)
                softmax_half(0)
                dm_half(1)
                s_half(0)
                softmax_half(1)
                s_half(1)
                squash(t)

    import os
    if not os.environ.get("BASS_SKIP_COMPILE"):
        nc.compile()
    return nc


def _host_prep(x, conv_w, conv_b, pcap_w, pcap_b, W):
    x = np.ascontiguousarray(np.asarray(x, np.float32))
    conv_w = np.asarray(conv_w, np.float32)
    conv_b = np.asarray(conv_b, np.float32)
    pcap_w = np.asarray(pcap_w, np.float32)
    pcap_b = np.asarray(pcap_b, np.float32)
    W = np.asarray(W, np.float32)

    w1t = np.ascontiguousarray(
        conv_w.transpose(2, 1, 3, 0).reshape(9, 27, 256)
    ).astype(np.float16)
    cb = np.ascontiguousarray(conv_b.reshape(256, 1))
    w2t = np.ascontiguousarray(
        pcap_w.transpose(1, 2, 3, 0).reshape(2, 128, 81, 256)
    ).astype(np.float16)
    pb = np.ascontiguousarray(pcap_b.reshape(1, 256))

    g = np.zeros((128, 16), np.float32)
    for p in range(128):
        g[p, p // 8] = 1.0
    g = g.astype(np.float16)
    blkmask = (np.arange(128)[:, None] // 16 ==
               np.arange(128)[None, :] // 16).astype(np.float16)
    ident = np.eye(128, dtype=np.float16)
    # repsel[d', (oh,d)] = delta(d==d')
    repsel = (np.arange(16)[:, None] == (np.arange(128)[None, :] % 16)
              ).astype(np.float16)

    W16 = W.astype(np.float16)
    shared = {"w1t": w1t, "cb": cb, "w2t": w2t, "pb": pb, "gmat": g,
              "blkmask": blkmask, "ident": ident, "repsel": repsel}
    in_maps = []
    for c in range(N_CORES):
        n_real = SIZES[c]
        o0 = OFFS[c]
        Wsh = np.zeros((NO, 2048, 16, 8), np.float16)
        Wsh[:n_real] = W16[o0:o0 + n_real]
        # wy[o, oh*16+d, co, k] = W[o, co*8+oh, d, k]
        a = Wsh.reshape(NO, 256, 8, 16, 8)          # [o, co, oh, d, k]
        wy = np.ascontiguousarray(
            a.transpose(0, 2, 3, 1, 4).reshape(NO, 128, 2048)).astype(np.float16)
        # ws[o, p, (h,oh,ow,d)] = W[o, (h*128+p)*8+oh, d, ow]
        b_ = Wsh.reshape(NO, 2, 128, 8, 16, 8)      # [o, h, p, oh, d, ow]
        wsx = b_.transpose(0, 2, 1, 3, 5, 4)        # [o, p, h, oh, ow, d]
        ws = np.ascontiguousarray(wsx.reshape(NO, 128, 2048)).astype(np.float16)
        # ws0[g, p, (j, o*16+d)]: chunk = g*16+j = (h,oh,ow)
        ws0a = wsx.transpose(2, 3, 4, 1, 0, 5).reshape(128, 128, NO * 16)
        ws0 = np.ascontiguousarray(
            ws0a.reshape(8, 16, 128, NO * 16).transpose(0, 2, 1, 3)
            .reshape(8, 128, 16 * NO * 16)).astype(np.float16)
        ob = np.where(np.arange(NO) < n_real, 0.0, -30.0).astype(np.float16)
        obias = np.ascontiguousarray(np.broadcast_to(ob, (128, NO))).copy()

        m = dict(shared)
        m["x_sh"] = np.ascontiguousarray(x[c * B:(c + 1) * B])
        m["wy"] = wy
        m["ws"] = ws
        m["ws0"] = ws0
        m["obias"] = obias.astype(np.float16)
        in_maps.append(m)
    return in_maps


def run(inputs, trace=False, **kw):
    key = "nc"
    if key not in _CACHE:
        _CACHE[key] = _build()
    nc = _CACHE[key]
    in_maps = _host_prep(**inputs)
    res = bass_utils.run_bass_kernel_spmd(
        nc, in_maps, core_ids=list(range(N_CORES)), trace=trace, **kw)
    return res


def kernel(**inputs):
    res = run(inputs)
    v = np.concatenate(
        [res.results[c]["v_out"][:, :SIZES[c], :] for c in range(N_CORES)],
        axis=1)
    return v


# revision 29
# speedup vs baseline: 1.0228x; 1.0228x over previous
"""CapsNet-CIFAR100 forward on 8 Trainium2 NeuronCores.

Hybrid sharding: conv stem data-parallel (8 images/core), dynamic routing
out-capsule-sharded (each core: all 64 images x 13 o-slots).  u_hat (26M
elems) is never materialized:

  pass 0:  s0 = 0.01 * sum_ik W u           (PE, (i,k)-contraction)
  dm path: y = sum_d W v  (PE, half of u_hat), dm = sum_k y*u (DVE folds)
  softmax over o is GLOBAL: z partial-summed per core, AllReduce'd.
  s path:  cu = c*u (DVE), s = sum_ik W cu  (PE, (i,k)-contraction)

Index maps (per core):  i = co*8+oh, k = ow;  b in [64];  o in [13] slots.
  P1 = oh*16 + b%16 (quads q=b//16)   "dm layout"  [p, (o, q, co)]
  P2 = co%128 (halves h=co//128)      "s layout"   [p, (h, oh, ow, b)]
conv2 runs channel-major (out partitions = co%128) so squashed u exports
contiguously; u AllGather (262KB/core) then one strided copy -> u_s, and
64 PE transposes -> u_y.  AllReduce of z=[128,1024] f16 per routing pass.
"""

from contextlib import ExitStack

import numpy as np
import concourse.bass as bass
import concourse.mybir as mybir
import concourse.tile as tile
from concourse import bacc
from concourse import bass_utils

F32 = mybir.dt.float32
F16 = mybir.dt.float16
AF = mybir.ActivationFunctionType
ALU = mybir.AluOpType
AX = mybir.AxisListType

N_CORES = 8
B = 8            # conv batch per core
NO = 13          # o-slots per core (padded)
SIZES = [13, 13, 13, 13, 12, 12, 12, 12]
OFFS = [0, 13, 26, 39, 52, 64, 76, 88]
EPS = 1e-8

_CACHE = {}


def _build():
    nc = bacc.Bacc("TRN2", target_bir_lowering=False, debug=False,
                   num_devices=N_CORES)
    RG = [list(range(N_CORES))]

    # conv inputs
    xd = nc.dram_tensor("x_sh", [B, 3, 32, 32], F32, kind="ExternalInput").ap()
    w1d = nc.dram_tensor("w1t", [9, 27, 256], F16, kind="ExternalInput").ap()
    cbd = nc.dram_tensor("cb", [256, 1], F32, kind="ExternalInput").ap()
    w2d = nc.dram_tensor("w2t", [2, 128, 81, 256], F16, kind="ExternalInput").ap()
    pbd = nc.dram_tensor("pb", [256, 1], F32, kind="ExternalInput").ap()
    # routing inputs
    wyd = nc.dram_tensor("wy", [NO, 128, 2048], F16, kind="ExternalInput").ap()
    wsd = nc.dram_tensor("ws", [NO, 128, 2048], F16, kind="ExternalInput").ap()
    ws0d = nc.dram_tensor("ws0", [8, 128, 16 * NO * 16], F16, kind="ExternalInput").ap()
    rsd = nc.dram_tensor("repsel", [16, 128], F16, kind="ExternalInput").ap()
    obd = nc.dram_tensor("obias", [128, NO], F16, kind="ExternalInput").ap()
    bmd = nc.dram_tensor("blkmask", [128, 128], F16, kind="ExternalInput").ap()
    idd = nc.dram_tensor("ident", [128, 128], F16, kind="ExternalInput").ap()
    # scratch / comm
    ubd_t = nc.dram_tensor("ub_d", [128 * 1024], F16, kind="Internal").ap()
    uall_t = nc.dram_tensor("uall_d", [8 * 128 * 1024], F16, kind="Internal",
                            addr_space="Shared").ap()
    zbd = nc.dram_tensor("zb_d", [128 * 1024], F16, kind="Internal").ap()
    zrd = nc.dram_tensor("zr_d", [128 * 1024], F16, kind="Internal",
                         addr_space="Shared").ap()
    vout = nc.dram_tensor("v_out", [64, NO, 16], F32, kind="ExternalOutput").ap()

    with tile.TileContext(nc) as tc:
        with ExitStack() as stack:
            cpool = stack.enter_context(tc.tile_pool(name="consts", bufs=1))

            # ---------- shared constants ----------
            w1sb = cpool.tile([27, 9, 256], F16, name="w1sb")
            nc.sync.dma_start(out=w1sb, in_=w1d.rearrange("k c o -> c k o"))
            cbsb = cpool.tile([128, 2, 1], F32, name="cbsb")
            nc.sync.dma_start(out=cbsb, in_=cbd.rearrange("(t p) one -> p t one", p=128))
            pb2 = cpool.tile([128, 2, 1], F32, name="pb2")
            nc.sync.dma_start(out=pb2, in_=pbd.rearrange("(t p) one -> p t one", p=128))
            epssb = cpool.tile([128, 1], F32, name="epssb")
            nc.vector.memset(epssb, EPS)
            bmsb = cpool.tile([128, 128], F16, name="bmsb")
            nc.sync.dma_start(out=bmsb, in_=bmd)
            idsb = cpool.tile([128, 128], F16, name="idsb")
            nc.sync.dma_start(out=idsb, in_=idd)
            obsb = cpool.tile([128, NO], F16, name="obsb")
            nc.sync.dma_start(out=obsb, in_=obd)
            rssb = cpool.tile([16, 128], F16, name="rssb")
            nc.sync.dma_start(out=rssb, in_=rsd)

            # ---------- conv stages ----------
            with tc.tile_pool(name="work", bufs=2) as wpool, \
                 tc.tile_pool(name="acts", bufs=1) as apool:
                # stage A: conv1 [B,3,32,32] -> h [256, B, 24, 24]
                with tc.tile_pool(name="hpool", bufs=1) as hpool:
                    hsb = [hpool.tile([128, B, 24, 24], F16, name="hsb",
                                      tag=f"h{c}") for c in range(2)]
                    with tc.tile_pool(name="imp", bufs=1) as impool, \
                         tc.tile_pool(name="psc", bufs=2, space="PSUM") as pscpool:
                        xsf = impool.tile([27, B, 32, 24], F32, name="xsf")
                        for ci in range(3):
                            for kw in range(9):
                                src = bass.AP(
                                    tensor=xd.tensor,
                                    offset=ci * 1024 + kw,
                                    ap=[[3072, B], [32, 32], [1, 24]],
                                )
                                nc.sync.dma_start(
                                    out=xsf[ci * 9 + kw:ci * 9 + kw + 1], in_=src)
                        xsb = impool.tile([27, B, 32, 24], F16, name="xsb")
                        nc.vector.tensor_copy(xsb, xsf)

                        for oc in range(2):
                            for b in range(B):
                                for hh in range(2):
                                    ph = pscpool.tile([128, 288], F32, name="ph",
                                                      tag="pconv")
                                    for kh in range(9):
                                        nc.tensor.matmul(
                                            ph,
                                            lhsT=w1sb[:, kh, oc * 128:(oc + 1) * 128],
                                            rhs=xsb[:, b, kh + hh * 12:
                                                    kh + hh * 12 + 12, :].rearrange(
                                                        "c h w -> c (h w)"),
                                            start=(kh == 0), stop=(kh == 8),
                                        )
                                    nc.scalar.activation(
                                        hsb[oc][:, b, hh * 12:(hh + 1) * 12, :].rearrange(
                                            "p h w -> p (h w)"),
                                        ph, AF.Relu, bias=cbsb[:, oc],
                                    )

                    # stage B: conv2 channel-major:
                    # p2c[hh][128 co, (b8, oh8, ow8)] = conv2 out + bias
                    p2c = [apool.tile([128, 8, 8, 8], F32, name="p2c",
                                      tag=f"p2c{hh}") for hh in range(2)]
                    with tc.tile_pool(name="w2", bufs=2) as w2pool, \
                         tc.tile_pool(name="psc2", bufs=1, space="PSUM") as psc2pool:
                        p2ps = [psc2pool.tile([128, 512], F32, name="p2ps",
                                              tag=f"p2ps{hh}") for hh in range(2)]
                        nmm = [0, 0]
                        for g in range(9):
                            w2g = [w2pool.tile([128, 9, 256], F16, name="w2g",
                                               tag="w2g") for _ in range(2)]
                            for cic in range(2):
                                nc.sync.dma_start(out=w2g[cic],
                                                  in_=w2d[cic, :, g * 9:(g + 1) * 9, :])
                            for j in range(9):
                                khw = g * 9 + j
                                kh, kw = khw // 9, khw % 9
                                for cic in range(2):
                                    rhs = hsb[cic][:, :, kh:kh + 16:2,
                                                   kw:kw + 16:2]
                                    for hh in range(2):
                                        nc.tensor.matmul(
                                            p2ps[hh],
                                            lhsT=w2g[cic][:, j,
                                                          hh * 128:(hh + 1) * 128],
                                            rhs=rhs,
                                            start=(nmm[hh] == 0),
                                            stop=(nmm[hh] == 161),
                                        )
                                        nmm[hh] += 1
                        for hh in range(2):
                            nc.scalar.activation(
                                p2c[hh].rearrange("p b g w -> p (b g w)"),
                                p2ps[hh], AF.Copy)
                            nc.vector.tensor_tensor(
                                out=p2c[hh], in0=p2c[hh],
                                in1=pb2[:, hh].unsqueeze(2).unsqueeze(3)
                                    .broadcast_to([128, 8, 8, 8]),
                                op=ALU.add)

                # stage C: squash over ow -> us_own [co, b8, h, oh, ow]
                us_own = apool.tile([128, 8, 2, 8, 8], F16, name="us_own")
                for hh in range(2):
                    sq = wpool.tile([128, 8, 8, 8], F32, name="sq", tag="sq")
                    nc.vector.tensor_mul(sq, p2c[hh], p2c[hh])
                    n2 = wpool.tile([128, 8, 8], F32, name="n2", tag="n2")
                    nc.vector.tensor_reduce(n2, sq, axis=AX.X, op=ALU.add)
                    r1 = wpool.tile([128, 8, 8], F32, name="r1", tag="r1")
                    nc.vector.tensor_scalar_add(r1, in0=n2, scalar1=1.0)
                    nc.vector.reciprocal(r1, r1)
                    qq = wpool.tile([128, 8, 8], F32, name="qq", tag="qq")
                    nc.scalar.activation(qq.rearrange("p b g -> p (b g)"),
                                         n2.rearrange("p b g -> p (b g)"),
                                         AF.Sqrt, bias=epssb)
                    nc.vector.reciprocal(qq, qq)
                    ff = wpool.tile([128, 8, 8], F32, name="ff", tag="ff")
                    nc.vector.tensor_mul(ff, n2, r1)
                    nc.vector.tensor_mul(ff, ff, qq)
                    nc.vector.tensor_tensor(
                        out=us_own[:, :, hh], in0=p2c[hh],
                        in1=ff.unsqueeze(3).broadcast_to([128, 8, 8, 8]),
                        op=ALU.mult)

                # export u: ub_d[co*1024 + b8*128 + hh*64... layout (co,b8,h,g,w)]
                nc.sync.dma_start(
                    out=bass.AP(tensor=ubd_t.tensor, offset=0,
                                ap=[[1024, 128], [1, 1024]]),
                    in_=us_own)

            # ---------- u AllGather ----------
            nc.gpsimd.collective_compute(
                "AllGather", ALU.bypass, replica_groups=RG,
                ins=[ubd_t.opt()], outs=[uall_t.opt()])

            # ---------- routing-persistent tiles + u relayouts ----------
            rpool = stack.enter_context(tc.tile_pool(name="rp", bufs=1))
            u_y = rpool.tile([128, 4, 256, 8], F16, name="u_y")
            u_s = rpool.tile([128, 2, 8, 8, 64], F16, name="u_s")
            with tc.tile_pool(name="rly", bufs=1) as rlpool, \
                 tc.tile_pool(name="rlps", bufs=4, space="PSUM") as rlps:
                # import gathered u: [core][co, b8, h, oh, ow]
                ust = rlpool.tile([128, 8, 8, 2, 8, 8], F16, name="ust")
                for core in range(N_CORES):
                    nc.sync.dma_start(
                        out=ust[:, core],
                        in_=bass.AP(tensor=uall_t.tensor,
                                    offset=core * 128 * 1024,
                                    ap=[[1024, 128], [1, 1024]]))
                # u_s[p, (h, oh, ow, b=(core,b8))] via strided copies (per h)
                for h in range(2):
                    nc.vector.tensor_copy(
                        u_s[:, h],
                        ust[:, :, :, h].rearrange("p c b g w -> p g w (c b)"))
                # u_mid[co, (h, k, q, oh, b16)] = u_s[co, h, oh, k, q*16+b16]
                umid = rlpool.tile([128, 2, 8, 4, 8, 16], F16, name="umid")
                for h in range(2):
                    nc.vector.tensor_copy(
                        umid[:, h],
                        u_s[:, h].rearrange("p oh k (q s) -> p k q oh s", q=4))
                # T2: u_y[p=(oh,b16), (q, co, k)]
                for h in range(2):
                    for k in range(8):
                        for q in range(4):
                            t2 = rlps.tile([128, 128], F16, name="t2", tag="t2")
                            nc.tensor.transpose(t2, umid[:, h, k, q].rearrange(
                                "p a s -> p (a s)"), idsb)
                            nc.scalar.activation(
                                u_y[:, q, h * 128:(h + 1) * 128, k],
                                t2, AF.Copy)

            dm0 = rpool.tile([128, NO, 4, 256], F16, name="dm0")
            dm1 = rpool.tile([128, NO, 4, 256], F16, name="dm1")
            vrep = rpool.tile([128, NO, 64], F16, name="vrep")
            s_all = rpool.tile([64, NO, 16], F32, name="s_all")
            zsum = rpool.tile([128, 1024], F16, name="zsum")
            zf32 = rpool.tile([128, 1024], F32, name="zf32")
            zi16 = rpool.tile([128, 4, 256], F16, name="zi16")
            zpa = rpool.tile([128, 4, 256], F16, name="zpa")
            zpb = rpool.tile([128, 4, 256], F16, name="zpb")
            zpc = rpool.tile([128, 4, 256], F16, name="zpc")

            vpool = stack.enter_context(tc.tile_pool(name="vp", bufs=1))
            vpsp = stack.enter_context(tc.tile_pool(name="vpsp", bufs=1, space="PSUM"))
            vpsp2 = stack.enter_context(tc.tile_pool(name="vpsp2", bufs=1, space="PSUM"))

            def squash(t):
                """s_all [64, NO, 16] f32 -> v; t=2 writes vout, else vrep."""
                sq = vpool.tile([64, NO, 16], F32, name="ssq", tag="ssq")
                nc.vector.tensor_mul(sq, s_all, s_all)
                n2 = vpool.tile([64, NO], F32, name="sn2", tag="sn2")
                nc.vector.tensor_reduce(n2, sq, axis=AX.X, op=ALU.add)
                r1 = vpool.tile([64, NO], F32, name="sr1", tag="sr1")
                nc.vector.tensor_scalar_add(r1, in0=n2, scalar1=1.0)
                nc.vector.reciprocal(r1, r1)
                qq = vpool.tile([64, NO], F32, name="sqq", tag="sqq")
                nc.scalar.activation(qq, n2, AF.Sqrt, bias=epssb[:64])
                nc.vector.reciprocal(qq, qq)
                ff = vpool.tile([64, NO], F32, name="sff", tag="sff")
                nc.vector.tensor_mul(ff, n2, r1)
                nc.vector.tensor_mul(ff, ff, qq)
                vv = vpool.tile([64, NO, 16], F32, name="svv", tag="svv")
                nc.vector.tensor_tensor(
                    out=vv, in0=s_all,
                    in1=ff.unsqueeze(2).broadcast_to([64, NO, 16]), op=ALU.mult)
                if t == 2:
                    nc.sync.dma_start(out=vout, in_=vv)
                else:
                    # vrep[p=(oh,d), (o,b)] via transpose + replication matmul
                    vb16 = vpool.tile([64, NO, 16], F16, name="svb", tag="svb")
                    nc.vector.tensor_copy(vb16, vv)
                    for o in range(NO):
                        tp = vpsp.tile([16, 64], F16, name="vtp", tag="vtp")
                        nc.tensor.transpose(tp, vb16[:, o, :], idsb[:64, :64])
                        vts = vpool.tile([16, 64], F16, name="vts", tag="vts")
                        nc.scalar.activation(vts, tp, AF.Copy)
                        rp = vpsp2.tile([128, 64], F32, name="vrp", tag="vrp")
                        nc.tensor.matmul(rp, lhsT=rssb, rhs=vts,
                                         start=True, stop=True)
                        nc.scalar.activation(vrep[:, o, :], rp, AF.Copy)

            # ---------- pass 0 ----------
            with tc.tile_pool(name="p0", bufs=2) as p0pool, \
                 tc.tile_pool(name="p0c", bufs=1) as p0cpool, \
                 tc.tile_pool(name="p0ps", bufs=1, space="PSUM") as p0psp:
                cu0 = p0cpool.tile([128, 2, 8, 8, 64], F16, name="cu0")
                nc.vector.tensor_scalar_mul(cu0, in0=u_s, scalar1=0.01)
                s0ps = p0psp.tile([64, NO * 16], F32, name="s0ps")
                for g in range(8):
                    w0t = p0pool.tile([128, 16, NO * 16], F16, name="w0t", tag="w0t")
                    nc.sync.dma_start(
                        out=w0t,
                        in_=bass.AP(tensor=ws0d.tensor, offset=g * 128 * 16 * NO * 16,
                                    ap=[[16 * NO * 16, 128], [1, 16 * NO * 16]]))
                    for j in range(16):
                        ch = g * 16 + j
                        h, oh, ow = ch // 64, (ch // 8) % 8, ch % 8
                        nc.tensor.matmul(
                            s0ps, lhsT=cu0[:, h, oh, ow, :], rhs=w0t[:, j, :],
                            start=(ch == 0), stop=(ch == 127))
                nc.scalar.activation(s_all.rearrange("b o d -> b (o d)"), s0ps,
                                     AF.Copy)
                squash(0)

            # ---------- passes 1, 2 ----------
            wypool = stack.enter_context(tc.tile_pool(name="wyp", bufs=2))
            wspool = stack.enter_context(tc.tile_pool(name="wsp", bufs=2))
            ypool = stack.enter_context(tc.tile_pool(name="yp", bufs=2))
            cupool = stack.enter_context(tc.tile_pool(name="cup", bufs=2))
            ctpool = stack.enter_context(tc.tile_pool(name="ctp", bufs=2))
            vbpool = stack.enter_context(tc.tile_pool(name="vbp", bufs=2))
            ypsp = stack.enter_context(tc.tile_pool(name="ypsp", bufs=2, space="PSUM"))
            tpsp = stack.enter_context(tc.tile_pool(name="tpsp", bufs=2, space="PSUM"))
            spsp = stack.enter_context(tc.tile_pool(name="spsp", bufs=2, space="PSUM"))

            for t in (1, 2):
                dmt = dm0 if t == 1 else dm1
                # ---- y / dm stage ----
                for o in range(NO):
                    wyt = wypool.tile([128, 2048], F16, name="wyt", tag="wyt")
                    nc.sync.dma_start(
                        out=wyt,
                        in_=bass.AP(tensor=wyd.tensor, offset=o * 128 * 2048,
                                    ap=[[2048, 128], [1, 2048]]))
                    vblk = vbpool.tile([128, 4, 8, 16], F16, name="vblk", tag="vblk")
                    nc.vector.tensor_tensor(
                        out=vblk,
                        in0=vrep[:, o, :].rearrange("p (q s) -> p q s", q=4)
                            .unsqueeze(2).broadcast_to([128, 4, 8, 16]),
                        in1=bmsb.rearrange("p (a s) -> p a s", a=8)
                            .unsqueeze(1).broadcast_to([128, 4, 8, 16]),
                        op=ALU.mult)
                    y16 = ypool.tile([128, 4, 256, 8], F16, name="y16", tag="y16")
                    for q in range(4):
                        for cc in range(4):
                            yps = ypsp.tile([128, 512], F32, name="yps", tag="yps")
                            nc.tensor.matmul(
                                yps,
                                lhsT=vblk[:, q].rearrange("p a s -> p (a s)"),
                                rhs=wyt[:, cc * 512:(cc + 1) * 512],
                                start=True, stop=True)
                            nc.scalar.activation(
                                y16[:, q, cc * 64:(cc + 1) * 64, :].rearrange(
                                    "p c k -> p (c k)"),
                                yps, AF.Copy)
                    nc.vector.tensor_tensor(out=y16, in0=y16, in1=u_y, op=ALU.mult)
                    eng = nc.gpsimd if o % 2 == 0 else nc.vector
                    eng.tensor_tensor(out=y16[:, :, :, 0:4], in0=y16[:, :, :, 0:4],
                                      in1=y16[:, :, :, 4:8], op=ALU.add)
                    nc.vector.tensor_tensor(out=y16[:, :, :, 0:2],
                                            in0=y16[:, :, :, 0:2],
                                            in1=y16[:, :, :, 2:4], op=ALU.add)
                    nc.vector.tensor_tensor(out=dmt[:, o], in0=y16[:, :, :, 0],
                                            in1=y16[:, :, :, 1], op=ALU.add)
                    if o == NO - 1:
                        nc.gpsimd.tensor_tensor(
                            out=dmt[:, NO - 1], in0=dmt[:, NO - 1],
                            in1=obsb[:, NO - 1:NO].unsqueeze(2)
                                .broadcast_to([128, 4, 256]),
                            op=ALU.add)

                if t == 2:
                    nc.vector.tensor_tensor(out=dm0, in0=dm0, in1=dm1, op=ALU.add)

                # e = exp(logits) into dm1; z = sum_o e (3-acc tree on DVE)
                nc.scalar.activation(dm1.rearrange("p o q c -> p (o q c)"),
                                     dm0.rearrange("p o q c -> p (o q c)"), AF.Exp)
                nc.vector.tensor_tensor(out=zpa, in0=dm1[:, 0], in1=dm1[:, 1],
                                        op=ALU.add)
                nc.vector.tensor_tensor(out=zpb, in0=dm1[:, 2], in1=dm1[:, 3],
                                        op=ALU.add)
                nc.vector.tensor_tensor(out=zpc, in0=dm1[:, 4], in1=dm1[:, 5],
                                        op=ALU.add)
                for o in range(6, NO, 3):
                    nc.vector.tensor_tensor(out=zpa, in0=zpa, in1=dm1[:, o],
                                            op=ALU.add)
                    if o + 1 < NO:
                        nc.vector.tensor_tensor(out=zpb, in0=zpb, in1=dm1[:, o + 1],
                                                op=ALU.add)
                    if o + 2 < NO:
                        nc.vector.tensor_tensor(out=zpc, in0=zpc, in1=dm1[:, o + 2],
                                                op=ALU.add)
                nc.vector.tensor_tensor(out=zpb, in0=zpb, in1=zpc, op=ALU.add)
                nc.vector.tensor_tensor(out=zpa, in0=zpa, in1=zpb, op=ALU.add)
                nc.sync.dma_start(
                    out=bass.AP(tensor=zbd.tensor, offset=0,
                                ap=[[1024, 128], [1, 1024]]),
                    in_=zpa.rearrange("p q c -> p (q c)"))
                nc.gpsimd.collective_compute(
                    "AllReduce", ALU.add, replica_groups=RG,
                    ins=[zbd.opt()], outs=[zrd.opt()])
                nc.sync.dma_start(
                    out=zsum,
                    in_=bass.AP(tensor=zrd.tensor, offset=0,
                                ap=[[1024, 128], [1, 1024]]))
                nc.vector.tensor_copy(zf32, zsum)
                nc.vector.reciprocal(zf32, zf32)
                nc.vector.tensor_copy(zi16.rearrange("p q c -> p (q c)"), zf32)
                nc.vector.tensor_tensor(
                    out=dm1, in0=dm1,
                    in1=zi16.unsqueeze(1).broadcast_to([128, NO, 4, 256]),
                    op=ALU.mult)

                # ---- s stage ----
                for o in range(NO):
                    wst = wspool.tile([128, 128, 16], F16, name="wst", tag="wst")
                    nc.sync.dma_start(
                        out=wst,
                        in_=bass.AP(tensor=wsd.tensor, offset=o * 128 * 2048,
                                    ap=[[2048, 128], [1, 2048]]))
                    ct = ctpool.tile([128, 2, 8, 16 * 4], F16, name="ct", tag="ct")
                    for qq_ in range(4):
                        for h in range(2):
                            tps = tpsp.tile([128, 128], F16, name="tps", tag="tps")
                            nc.tensor.transpose(
                                tps, dm1[:, o, qq_, h * 128:(h + 1) * 128], idsb)
                            if (qq_ + h) % 2 == 0:
                                nc.vector.tensor_copy(
                                    ct[:, h, :, qq_ * 16:(qq_ + 1) * 16],
                                    tps.rearrange("p (a s) -> p a s", a=8))
                            else:
                                nc.scalar.activation(
                                    ct[:, h, :, qq_ * 16:(qq_ + 1) * 16],
                                    tps.rearrange("p (a s) -> p a s", a=8), AF.Copy)
                    cu = cupool.tile([128, 2, 8, 8, 64], F16, name="cu", tag="cu")
                    nc.vector.tensor_tensor(
                        out=cu, in0=u_s,
                        in1=ct.unsqueeze(3).broadcast_to([128, 2, 8, 8, 64]),
                        op=ALU.mult)
                    sps = spsp.tile([64, 16], F32, name="sps", tag="sps")
                    for ch in range(128):
                        h, oh, ow = ch // 64, (ch // 8) % 8, ch % 8
                        nc.tensor.matmul(
                            sps, lhsT=cu[:, h, oh, ow, :],
                            rhs=wst[:, ch, :],
                            start=(ch == 0), stop=(ch == 127))
                    nc.scalar.activation(s_all[:, o, :], sps, AF.Copy)
                squash(t)

    import os
    if not os.environ.get("BASS_SKIP_COMPILE"):
        nc.compile()
    return nc


def _host_prep(x, conv_w, conv_b, pcap_w, pcap_b, W):
    x = np.ascontiguousarray(np.asarray(x, np.float32))
    conv_w = np.asarray(conv_w, np.float32)
    conv_b = np.asarray(conv_b, np.float32)
    pcap_w = np.asarray(pcap_w, np.float32)
    pcap_b = np.asarray(pcap_b, np.float32)
    W = np.asarray(W, np.float32)

    w1t = np.ascontiguousarray(
        conv_w.transpose(2, 1, 3, 0).reshape(9, 27, 256)
    ).astype(np.float16)
    cb = np.ascontiguousarray(conv_b.reshape(256, 1))
    w2t = np.ascontiguousarray(
        pcap_w.transpose(1, 2, 3, 0).reshape(2, 128, 81, 256)
    ).astype(np.float16)
    pb = np.ascontiguousarray(pcap_b.reshape(256, 1))

    blkmask = (np.arange(128)[:, None] // 16 ==
               np.arange(128)[None, :] // 16).astype(np.float16)
    ident = np.eye(128, dtype=np.float16)
    repsel = (np.arange(16)[:, None] == (np.arange(128)[None, :] % 16)
              ).astype(np.float16)

    W16 = W.astype(np.float16)
    shared = {"w1t": w1t, "cb": cb, "w2t": w2t, "pb": pb,
              "blkmask": blkmask, "ident": ident, "repsel": repsel}
    in_maps = []
    for c in range(N_CORES):
        n_real = SIZES[c]
        o0 = OFFS[c]
        Wsh = np.zeros((NO, 2048, 16, 8), np.float16)
        Wsh[:n_real] = W16[o0:o0 + n_real]
        # wy[o, oh*16+d, co, k] = W[o, co*8+oh, d, k]
        a = Wsh.reshape(NO, 256, 8, 16, 8)          # [o, co, oh, d, k]
        wy = np.ascontiguousarray(
            a.transpose(0, 2, 3, 1, 4).reshape(NO, 128, 2048)).astype(np.float16)
        # ws[o, p, (h,oh,ow,d)] = W[o, (h*128+p)*8+oh, d, ow]
        b_ = Wsh.reshape(NO, 2, 128, 8, 16, 8)      # [o, h, p, oh, d, ow]
        wsx = b_.transpose(0, 2, 1, 3, 5, 4)        # [o, p, h, oh, ow, d]
        ws = np.ascontiguousarray(wsx.reshape(NO, 128, 2048)).astype(np.float16)
        # ws0[g, p, (j, o*16+d)]: chunk = g*16+j = (h,oh,ow)
        ws0a = wsx.transpose(2, 3, 4, 1, 0, 5).reshape(128, 128, NO * 16)
        ws0 = np.ascontiguousarray(
            ws0a.reshape(8, 16, 128, NO * 16).transpose(0, 2, 1, 3)
            .reshape(8, 128, 16 * NO * 16)).astype(np.float16)
        ob = np.where(np.arange(NO) < n_real, 0.0, -30.0).astype(np.float16)
        obias = np.ascontiguousarray(np.broadcast_to(ob, (128, NO))).copy()

        m = dict(shared)
        m["x_sh"] = np.ascontiguousarray(x[c * B:(c + 1) * B])
        m["wy"] = wy
        m["ws"] = ws
        m["ws0"] = ws0
        m["obias"] = obias.astype(np.float16)
        in_maps.append(m)
    return in_maps


def run(inputs, trace=False, **kw):
    key = "nc"
    if key not in _CACHE:
        _CACHE[key] = _build()
    nc = _CACHE[key]
    in_maps = _host_prep(**inputs)
    res = bass_utils.run_bass_kernel_spmd(
        nc, in_maps, core_ids=list(range(N_CORES)), trace=trace, **kw)
    return res


def kernel(**inputs):
    res = run(inputs)
    v = np.concatenate(
        [res.results[c]["v_out"][:, :SIZES[c], :] for c in range(N_CORES)],
        axis=1)
    return v


# revision 36
# speedup vs baseline: 1.2756x; 1.2473x over previous
"""CapsNet-CIFAR100 forward on 8 Trainium2 NeuronCores.

Hybrid sharding: conv stem data-parallel (8 images/core), dynamic routing
out-capsule-sharded (each core: all 64 images x 13 o-slots).  u_hat (26M
elems) is never materialized:

  pass 0:  s0 = 0.01 * sum_ik W u           (PE, (i,k)-contraction)
  dm path: y = sum_d W v  (PE, half of u_hat), dm = sum_k y*u (DVE folds)
  softmax over o is GLOBAL: z partial-summed per core, AllReduce'd.
  s path:  cu = c*u (DVE), s = sum_ik W cu  (PE, (i,k)-contraction)

Index maps (per core):  i = co*8+oh, k = ow;  b in [64];  o in [13] slots.
  P1 = oh*16 + b%16 (quads q=b//16)   "dm layout"  [p, (o, q, co)]
  P2 = co%128 (halves h=co//128)      "s layout"   [p, (h, oh, ow, b)]
conv2 runs channel-major (out partitions = co%128) so squashed u exports
contiguously; u AllGather (262KB/core) then one strided copy -> u_s, and
64 PE transposes -> u_y.  AllReduce of z=[128,1024] f16 per routing pass.
"""

from contextlib import ExitStack

import numpy as np
import concourse.bass as bass
import concourse.mybir as mybir
import concourse.tile as tile
from concourse import bacc
from concourse import bass_utils

F32 = mybir.dt.float32
F16 = mybir.dt.float16
AF = mybir.ActivationFunctionType
ALU = mybir.AluOpType
AX = mybir.AxisListType

N_CORES = 8
B = 8            # conv batch per core
NO = 13          # o-slots per core (padded)
SIZES = [13, 13, 13, 13, 12, 12, 12, 12]
OFFS = [0, 13, 26, 39, 52, 64, 76, 88]
EPS = 1e-8

_CACHE = {}


def _build():
    nc = bacc.Bacc("TRN2", target_bir_lowering=False, debug=False,
                   num_devices=N_CORES)
    RG = [list(range(N_CORES))]

    # conv inputs
    xd = nc.dram_tensor("x_sh", [B, 3, 32, 32], F32, kind="ExternalInput").ap()
    w1d = nc.dram_tensor("w1t", [9, 27, 256], F16, kind="ExternalInput").ap()
    cbd = nc.dram_tensor("cb", [256, 1], F32, kind="ExternalInput").ap()
    w2d = nc.dram_tensor("w2t", [2, 128, 81, 256], F16, kind="ExternalInput").ap()
    pbd = nc.dram_tensor("pb", [256, 1], F32, kind="ExternalInput").ap()
    # routing inputs
    wyd = nc.dram_tensor("wy", [NO, 128, 2048], F16, kind="ExternalInput").ap()
    wsd = nc.dram_tensor("ws", [NO, 128, 2048], F16, kind="ExternalInput").ap()
    ws0d = nc.dram_tensor("ws0", [8, 128, 16 * NO * 16], F16, kind="ExternalInput").ap()
    rsd = nc.dram_tensor("repsel", [16, 128], F16, kind="ExternalInput").ap()
    obd = nc.dram_tensor("obias", [128, NO], F16, kind="ExternalInput").ap()
    bmd = nc.dram_tensor("blkmask", [128, 128], F16, kind="ExternalInput").ap()
    idd = nc.dram_tensor("ident", [128, 128], F16, kind="ExternalInput").ap()
    # scratch / comm
    ubd_t = nc.dram_tensor("ub_d", [128 * 1024], F16, kind="Internal").ap()
    uall_t = nc.dram_tensor("uall_d", [8 * 128 * 1024], F16, kind="Internal",
                            addr_space="Shared").ap()
    zbd = nc.dram_tensor("zb_d", [128 * 1024], F16, kind="Internal").ap()
    zrd = nc.dram_tensor("zr_d", [128 * 1024], F16, kind="Internal",
                         addr_space="Shared").ap()
    vout = nc.dram_tensor("v_out", [64, NO, 16], F32, kind="ExternalOutput").ap()

    with tile.TileContext(nc) as tc:
        with ExitStack() as stack:
            cpool = stack.enter_context(tc.tile_pool(name="consts", bufs=1))

            # ---------- shared constants ----------
            w1sb = cpool.tile([27, 9, 256], F16, name="w1sb")
            nc.sync.dma_start(out=w1sb, in_=w1d.rearrange("k c o -> c k o"))
            cbsb = cpool.tile([128, 2, 1], F32, name="cbsb")
            nc.sync.dma_start(out=cbsb, in_=cbd.rearrange("(t p) one -> p t one", p=128))
            pb2 = cpool.tile([128, 2, 1], F32, name="pb2")
            nc.sync.dma_start(out=pb2, in_=pbd.rearrange("(t p) one -> p t one", p=128))
            epssb = cpool.tile([128, 1], F32, name="epssb")
            nc.vector.memset(epssb, EPS)
            bmsb = cpool.tile([128, 128], F16, name="bmsb")
            nc.sync.dma_start(out=bmsb, in_=bmd)
            idsb = cpool.tile([128, 128], F16, name="idsb")
            nc.sync.dma_start(out=idsb, in_=idd)
            obsb = cpool.tile([128, NO], F16, name="obsb")
            nc.sync.dma_start(out=obsb, in_=obd)
            rssb = cpool.tile([16, 128], F16, name="rssb")
            nc.sync.dma_start(out=rssb, in_=rsd)

            # ---------- conv stages ----------
            with tc.tile_pool(name="work", bufs=2) as wpool, \
                 tc.tile_pool(name="acts", bufs=1) as apool:
                # stage A: conv1 [B,3,32,32] -> h [256, B, 24, 24]
                with tc.tile_pool(name="hpool", bufs=1) as hpool:
                    hsb = [hpool.tile([128, B, 24, 24], F16, name="hsb",
                                      tag=f"h{c}") for c in range(2)]
                    with tc.tile_pool(name="imp", bufs=1) as impool, \
                         tc.tile_pool(name="psc", bufs=2, space="PSUM") as pscpool:
                        xsf = impool.tile([27, B, 32, 24], F32, name="xsf")
                        for ci in range(3):
                            for kw in range(9):
                                src = bass.AP(
                                    tensor=xd.tensor,
                                    offset=ci * 1024 + kw,
                                    ap=[[3072, B], [32, 32], [1, 24]],
                                )
                                nc.sync.dma_start(
                                    out=xsf[ci * 9 + kw:ci * 9 + kw + 1], in_=src)
                        xsb = impool.tile([27, B, 32, 24], F16, name="xsb")
                        nc.vector.tensor_copy(xsb, xsf)

                        for oc in range(2):
                            for b in range(B):
                                for hh in range(2):
                                    ph = pscpool.tile([128, 288], F32, name="ph",
                                                      tag="pconv")
                                    for kh in range(9):
                                        nc.tensor.matmul(
                                            ph,
                                            lhsT=w1sb[:, kh, oc * 128:(oc + 1) * 128],
                                            rhs=xsb[:, b, kh + hh * 12:
                                                    kh + hh * 12 + 12, :].rearrange(
                                                        "c h w -> c (h w)"),
                                            start=(kh == 0), stop=(kh == 8),
                                        )
                                    nc.scalar.activation(
                                        hsb[oc][:, b, hh * 12:(hh + 1) * 12, :].rearrange(
                                            "p h w -> p (h w)"),
                                        ph, AF.Relu, bias=cbsb[:, oc],
                                    )

                    # stage B: conv2 channel-major:
                    # p2c[hh][128 co, (b8, oh8, ow8)] = conv2 out + bias
                    p2c = [apool.tile([128, 8, 8, 8], F32, name="p2c",
                                      tag=f"p2c{hh}") for hh in range(2)]
                    with tc.tile_pool(name="w2", bufs=2) as w2pool, \
                         tc.tile_pool(name="psc2", bufs=1, space="PSUM") as psc2pool:
                        p2ps = [psc2pool.tile([128, 512], F32, name="p2ps",
                                              tag=f"p2ps{hh}") for hh in range(2)]
                        nmm = [0, 0]
                        for g in range(9):
                            w2g = [w2pool.tile([128, 9, 256], F16, name="w2g",
                                               tag="w2g") for _ in range(2)]
                            for cic in range(2):
                                nc.sync.dma_start(out=w2g[cic],
                                                  in_=w2d[cic, :, g * 9:(g + 1) * 9, :])
                            for j in range(9):
                                khw = g * 9 + j
                                kh, kw = khw // 9, khw % 9
                                for cic in range(2):
                                    hshift = wpool.tile([128, B, 8, 8], F16,
                                                        name="hshift", tag="hshift")
                                    if cic == 0:
                                        nc.vector.tensor_copy(
                                            hshift,
                                            hsb[cic][:, :, kh:kh + 16:2, kw:kw + 16:2])
                                    else:
                                        nc.scalar.copy(
                                            hshift,
                                            hsb[cic][:, :, kh:kh + 16:2, kw:kw + 16:2])
                                    rhs = hshift.rearrange("p b h w -> p (b h w)")
                                    for hh in range(2):
                                        nc.tensor.matmul(
                                            p2ps[hh],
                                            lhsT=w2g[cic][:, j,
                                                          hh * 128:(hh + 1) * 128],
                                            rhs=rhs,
                                            start=(nmm[hh] == 0),
                                            stop=(nmm[hh] == 161),
                                        )
                                        nmm[hh] += 1
                        for hh in range(2):
                            nc.scalar.activation(
                                p2c[hh].rearrange("p b g w -> p (b g w)"),
                                p2ps[hh], AF.Copy)
                            nc.vector.tensor_tensor(
                                out=p2c[hh], in0=p2c[hh],
                                in1=pb2[:, hh].unsqueeze(2).unsqueeze(3)
                                    .broadcast_to([128, 8, 8, 8]),
                                op=ALU.add)

                # stage C: squash over ow -> us_own [co, b8, h, oh, ow]
                us_own = apool.tile([128, 8, 2, 8, 8], F16, name="us_own")
                for hh in range(2):
                    sq = wpool.tile([128, 8, 8, 8], F32, name="sq", tag="sq")
                    nc.vector.tensor_mul(sq, p2c[hh], p2c[hh])
                    n2 = wpool.tile([128, 8, 8], F32, name="n2", tag="n2")
                    nc.vector.tensor_reduce(n2, sq, axis=AX.X, op=ALU.add)
                    r1 = wpool.tile([128, 8, 8], F32, name="r1", tag="r1")
                    nc.vector.tensor_scalar_add(r1, in0=n2, scalar1=1.0)
                    nc.vector.reciprocal(r1, r1)
                    qq = wpool.tile([128, 8, 8], F32, name="qq", tag="qq")
                    nc.scalar.activation(qq.rearrange("p b g -> p (b g)"),
                                         n2.rearrange("p b g -> p (b g)"),
                                         AF.Sqrt, bias=epssb)
                    nc.vector.reciprocal(qq, qq)
                    ff = wpool.tile([128, 8, 8], F32, name="ff", tag="ff")
                    nc.vector.tensor_mul(ff, n2, r1)
                    nc.vector.tensor_mul(ff, ff, qq)
                    nc.vector.tensor_tensor(
                        out=us_own[:, :, hh], in0=p2c[hh],
                        in1=ff.unsqueeze(3).broadcast_to([128, 8, 8, 8]),
                        op=ALU.mult)

                # export u: ub_d[co*1024 + b8*128 + hh*64... layout (co,b8,h,g,w)]
                nc.sync.dma_start(
                    out=bass.AP(tensor=ubd_t.tensor, offset=0,
                                ap=[[1024, 128], [1, 1024]]),
                    in_=us_own)

            # ---------- u AllGather ----------
            nc.gpsimd.collective_compute(
                "AllGather", ALU.bypass, replica_groups=RG,
                ins=[ubd_t.opt()], outs=[uall_t.opt()])

            # ---------- routing-persistent tiles + u relayouts ----------
            rpool = stack.enter_context(tc.tile_pool(name="rp", bufs=1))
            u_y = rpool.tile([128, 4, 256, 8], F16, name="u_y")
            u_s = rpool.tile([128, 2, 8, 8, 64], F16, name="u_s")
            with tc.tile_pool(name="rly", bufs=1) as rlpool, \
                 tc.tile_pool(name="rlps", bufs=4, space="PSUM") as rlps:
                # import gathered u: [core][co, b8, h, oh, ow]
                ust = rlpool.tile([128, 8, 8, 2, 8, 8], F16, name="ust")
                for core in range(N_CORES):
                    nc.sync.dma_start(
                        out=ust[:, core],
                        in_=bass.AP(tensor=uall_t.tensor,
                                    offset=core * 128 * 1024,
                                    ap=[[1024, 128], [1, 1024]]))
                # u_s[p, (h, oh, ow, b=(core,b8))] via strided copies (per h)
                for h in range(2):
                    nc.vector.tensor_copy(
                        u_s[:, h],
                        ust[:, :, :, h].rearrange("p c b g w -> p g w (c b)"))
                # u_mid[co, (h, k, q, oh, b16)] = u_s[co, h, oh, k, q*16+b16]
                umid = rlpool.tile([128, 2, 8, 4, 8, 16], F16, name="umid")
                for h in range(2):
                    nc.vector.tensor_copy(
                        umid[:, h],
                        u_s[:, h].rearrange("p oh k (q s) -> p k q oh s", q=4))
                # T2: u_y[p=(oh,b16), (q, co, k)]
                for h in range(2):
                    for k in range(8):
                        for q in range(4):
                            t2 = rlps.tile([128, 128], F16, name="t2", tag="t2")
                            nc.tensor.transpose(t2, umid[:, h, k, q].rearrange(
                                "p a s -> p (a s)"), idsb)
                            nc.scalar.activation(
                                u_y[:, q, h * 128:(h + 1) * 128, k],
                                t2, AF.Copy)

            dm0 = rpool.tile([128, NO, 4, 256], F16, name="dm0")
            dm1 = rpool.tile([128, NO, 4, 256], F16, name="dm1")
            vrep = rpool.tile([128, NO, 64], F16, name="vrep")
            s_all = rpool.tile([64, NO, 16], F32, name="s_all")
            zi16 = rpool.tile([128, 4, 256], F16, name="zi16")
            zpa = rpool.tile([128, 4, 256], F16, name="zpa")
            zpb = rpool.tile([128, 4, 256], F16, name="zpb")

            vpool = stack.enter_context(tc.tile_pool(name="vp", bufs=1))
            vpsp = stack.enter_context(tc.tile_pool(name="vpsp", bufs=1, space="PSUM"))
            vpsp2 = stack.enter_context(tc.tile_pool(name="vpsp2", bufs=1, space="PSUM"))

            def squash(t):
                """s_all [64, NO, 16] f32 -> v; t=2 writes vout, else vrep."""
                sq = vpool.tile([64, NO, 16], F32, name="ssq", tag="ssq")
                nc.vector.tensor_mul(sq, s_all, s_all)
                n2 = vpool.tile([64, NO], F32, name="sn2", tag="sn2")
                nc.vector.tensor_reduce(n2, sq, axis=AX.X, op=ALU.add)
                r1 = vpool.tile([64, NO], F32, name="sr1", tag="sr1")
                nc.vector.tensor_scalar_add(r1, in0=n2, scalar1=1.0)
                nc.vector.reciprocal(r1, r1)
                qq = vpool.tile([64, NO], F32, name="sqq", tag="sqq")
                nc.scalar.activation(qq, n2, AF.Sqrt, bias=epssb[:64])
                nc.vector.reciprocal(qq, qq)
                ff = vpool.tile([64, NO], F32, name="sff", tag="sff")
                nc.vector.tensor_mul(ff, n2, r1)
                nc.vector.tensor_mul(ff, ff, qq)
                vv = vpool.tile([64, NO, 16], F32, name="svv", tag="svv")
                nc.vector.tensor_tensor(
                    out=vv, in0=s_all,
                    in1=ff.unsqueeze(2).broadcast_to([64, NO, 16]), op=ALU.mult)
                if t == 2:
                    nc.sync.dma_start(out=vout, in_=vv)
                else:
                    # vrep[p=(oh,d), (o,b)] via transpose + replication matmul
                    vb16 = vpool.tile([64, NO, 16], F16, name="svb", tag="svb")
                    nc.vector.tensor_copy(vb16, vv)
                    for o in range(NO):
                        tp = vpsp.tile([16, 64], F16, name="vtp", tag="vtp")
                        nc.tensor.transpose(tp, vb16[:, o, :], idsb[:64, :64])
                        vts = vpool.tile([16, 64], F16, name="vts", tag="vts")
                        nc.scalar.activation(vts, tp, AF.Copy)
                        rp = vpsp2.tile([128, 64], F32, name="vrp", tag="vrp")
                        nc.tensor.matmul(rp, lhsT=rssb, rhs=vts,
                                         start=True, stop=True)
                        nc.scalar.activation(vrep[:, o, :], rp, AF.Copy)

            # ---------- pass 0 ----------
            with tc.tile_pool(name="p0", bufs=2) as p0pool, \
                 tc.tile_pool(name="p0c", bufs=1) as p0cpool, \
                 tc.tile_pool(name="p0ps", bufs=1, space="PSUM") as p0psp:
                cu0 = p0cpool.tile([128, 2, 8, 8, 64], F16, name="cu0")
                nc.vector.tensor_scalar_mul(cu0, in0=u_s, scalar1=0.01)
                s0ps = p0psp.tile([64, NO * 16], F32, name="s0ps")
                for g in range(8):
                    w0t = p0pool.tile([128, 16, NO * 16], F16, name="w0t", tag="w0t")
                    nc.sync.dma_start(
                        out=w0t,
                        in_=bass.AP(tensor=ws0d.tensor, offset=g * 128 * 16 * NO * 16,
                                    ap=[[16 * NO * 16, 128], [1, 16 * NO * 16]]))
                    for j in range(16):
                        ch = g * 16 + j
                        h, oh, ow = ch // 64, (ch // 8) % 8, ch % 8
                        nc.tensor.matmul(
                            s0ps, lhsT=cu0[:, h, oh, ow, :], rhs=w0t[:, j, :],
                            start=(ch == 0), stop=(ch == 127))
                nc.scalar.activation(s_all.rearrange("b o d -> b (o d)"), s0ps,
                                     AF.Copy)
                squash(0)

            # ---------- passes 1, 2 ----------
            wypool = stack.enter_context(tc.tile_pool(name="wyp", bufs=2))
            wspool = stack.enter_context(tc.tile_pool(name="wsp", bufs=2))
            ypool = stack.enter_context(tc.tile_pool(name="yp", bufs=2))
            cupool = stack.enter_context(tc.tile_pool(name="cup", bufs=2))
            ctpool = stack.enter_context(tc.tile_pool(name="ctp", bufs=3))
            vbpool = stack.enter_context(tc.tile_pool(name="vbp", bufs=2))
            ypsp = stack.enter_context(tc.tile_pool(name="ypsp", bufs=2, space="PSUM"))
            tpsp = stack.enter_context(tc.tile_pool(name="tpsp", bufs=2, space="PSUM"))
            spsp = stack.enter_context(tc.tile_pool(name="spsp", bufs=2, space="PSUM"))

            uz = rpool.tile([128, 2, 8, 8, 64], F16, name="uz")
            ziT = rpool.tile([128, 2, 8, 64], F16, name="ziT")

            for t in (1, 2):
                dmt = dm0 if t == 1 else dm1
                # ---- y / dm stage (exp + z-accum fused per o) ----
                for o in range(NO):
                    wyt = wypool.tile([128, 2048], F16, name="wyt", tag="wyt")
                    nc.sync.dma_start(
                        out=wyt,
                        in_=bass.AP(tensor=wyd.tensor, offset=o * 128 * 2048,
                                    ap=[[2048, 128], [1, 2048]]))
                    vblk = vbpool.tile([128, 4, 8, 16], F16, name="vblk", tag="vblk")
                    nc.vector.tensor_tensor(
                        out=vblk,
                        in0=vrep[:, o, :].rearrange("p (q s) -> p q s", q=4)
                            .unsqueeze(2).broadcast_to([128, 4, 8, 16]),
                        in1=bmsb.rearrange("p (a s) -> p a s", a=8)
                            .unsqueeze(1).broadcast_to([128, 4, 8, 16]),
                        op=ALU.mult)
                    y16 = ypool.tile([128, 4, 256, 8], F16, name="y16", tag="y16")
                    for q in range(4):
                        for cc in range(4):
                            yps = ypsp.tile([128, 512], F32, name="yps", tag="yps")
                            nc.tensor.matmul(
                                yps,
                                lhsT=vblk[:, q].rearrange("p a s -> p (a s)"),
                                rhs=wyt[:, cc * 512:(cc + 1) * 512],
                                start=True, stop=True)
                            dst = y16[:, q, cc * 64:(cc + 1) * 64, :].rearrange(
                                "p c k -> p (c k)")
                            if (q * 4 + cc) % 5 == 4:
                                nc.vector.tensor_copy(dst, yps)
                            else:
                                nc.scalar.activation(dst, yps, AF.Copy)
                    nc.vector.tensor_tensor(out=y16, in0=y16, in1=u_y, op=ALU.mult)
                    eng = nc.gpsimd if o % 2 == 0 else nc.vector
                    eng.tensor_tensor(out=y16[:, :, :, 0:4], in0=y16[:, :, :, 0:4],
                                      in1=y16[:, :, :, 4:8], op=ALU.add)
                    nc.vector.tensor_tensor(out=y16[:, :, :, 0:2],
                                            in0=y16[:, :, :, 0:2],
                                            in1=y16[:, :, :, 2:4], op=ALU.add)
                    nc.vector.tensor_tensor(out=dmt[:, o], in0=y16[:, :, :, 0],
                                            in1=y16[:, :, :, 1], op=ALU.add)
                    if o == NO - 1:
                        nc.gpsimd.tensor_tensor(
                            out=dmt[:, NO - 1], in0=dmt[:, NO - 1],
                            in1=obsb[:, NO - 1:NO].unsqueeze(2)
                                .broadcast_to([128, 4, 256]),
                            op=ALU.add)
                    if t == 2:
                        nc.vector.tensor_tensor(out=dm0[:, o], in0=dm0[:, o],
                                                in1=dm1[:, o], op=ALU.add)
                    # e_o = exp(logits_o) into dm1[:, o];  z-accum on gpsimd
                    nc.scalar.activation(dm1[:, o].rearrange("p q c -> p (q c)"),
                                         dm0[:, o].rearrange("p q c -> p (q c)"),
                                         AF.Exp)
                    acc = (zpa, zpb)[o % 2]
                    if o < 2:
                        nc.gpsimd.tensor_copy(acc, dm1[:, o])
                    else:
                        nc.gpsimd.tensor_tensor(out=acc, in0=acc, in1=dm1[:, o],
                                                op=ALU.add)
                # z = zpa+zpb -> AllReduce -> zi
                nc.vector.tensor_tensor(out=zpa, in0=zpa, in1=zpb, op=ALU.add)
                nc.sync.dma_start(
                    out=bass.AP(tensor=zbd.tensor, offset=0,
                                ap=[[1024, 128], [1, 1024]]),
                    in_=zpa.rearrange("p q c -> p (q c)"))
                nc.gpsimd.collective_compute(
                    "AllReduce", ALU.add, replica_groups=RG,
                    ins=[zbd.opt()], outs=[zrd.opt()])
                nc.sync.dma_start(
                    out=zpa.rearrange("p q c -> p (q c)"),
                    in_=bass.AP(tensor=zrd.tensor, offset=0,
                                ap=[[1024, 128], [1, 1024]]))

                # ---- s stage ----
                # e-transposes only need e: first few run during the AllReduce
                cts = {}

                def emit_ct(o):
                    ct = ctpool.tile([128, 2, 8, 16 * 4], F16, name="ct", tag="ct")
                    for qq_ in range(4):
                        for h in range(2):
                            tps = tpsp.tile([128, 128], F16, name="tps", tag="tps")
                            nc.tensor.transpose(
                                tps, dm1[:, o, qq_, h * 128:(h + 1) * 128], idsb)
                            nc.scalar.activation(
                                ct[:, h, :, qq_ * 16:(qq_ + 1) * 16],
                                tps.rearrange("p (a s) -> p a s", a=8), AF.Copy)
                    cts[o] = ct

                for o in range(2):
                    emit_ct(o)
                # zi = 1/z, transposed to [co, (h, oh, b)], folded into u_s
                with nc.allow_low_precision(reason="z~O(100), f16 recip err 1e-3 ok"):
                    nc.vector.reciprocal(zi16.rearrange("p q c -> p (q c)"),
                                         zpa.rearrange("p q c -> p (q c)"))
                for q in range(4):
                    for h in range(2):
                        tps = tpsp.tile([128, 128], F16, name="tps", tag="tps")
                        nc.tensor.transpose(
                            tps, zi16[:, q, h * 128:(h + 1) * 128], idsb)
                        nc.scalar.activation(
                            ziT[:, h, :, q * 16:(q + 1) * 16],
                            tps.rearrange("p (a s) -> p a s", a=8), AF.Copy)
                nc.vector.tensor_tensor(
                    out=uz, in0=u_s,
                    in1=ziT.unsqueeze(3).broadcast_to([128, 2, 8, 8, 64]),
                    op=ALU.mult)
                for o in range(NO):
                    wst = wspool.tile([128, 128, 16], F16, name="wst", tag="wst")
                    nc.sync.dma_start(
                        out=wst,
                        in_=bass.AP(tensor=wsd.tensor, offset=o * 128 * 2048,
                                    ap=[[2048, 128], [1, 2048]]))
                    if o + 2 < NO:
                        emit_ct(o + 2)
                    ct = cts.pop(o)
                    cu = cupool.tile([128, 2, 8, 8, 64], F16, name="cu", tag="cu")
                    nc.vector.tensor_tensor(
                        out=cu, in0=uz,
                        in1=ct.unsqueeze(3).broadcast_to([128, 2, 8, 8, 64]),
                        op=ALU.mult)
                    sps = spsp.tile([64, 16], F32, name="sps", tag="sps")
                    for ch in range(128):
                        h, oh, ow = ch // 64, (ch // 8) % 8, ch % 8
                        nc.tensor.matmul(
                            sps, lhsT=cu[:, h, oh, ow, :],
                            rhs=wst[:, ch, :],
                            start=(ch == 0), stop=(ch == 127))
                    nc.scalar.activation(s_all[:, o, :], sps, AF.Copy)
                squash(t)

    import os
    if not os.environ.get("BASS_SKIP_COMPILE"):
        nc.compile()
    return nc


def _host_prep(x, conv_w, conv_b, pcap_w, pcap_b, W):
    x = np.ascontiguousarray(np.asarray(x, np.float32))
    conv_w = np.asarray(conv_w, np.float32)
    conv_b = np.asarray(conv_b, np.float32)
    pcap_w = np.asarray(pcap_w, np.float32)
    pcap_b = np.asarray(pcap_b, np.float32)
    W = np.asarray(W, np.float32)

    w1t = np.ascontiguousarray(
        conv_w.transpose(2, 1, 3, 0).reshape(9, 27, 256)
    ).astype(np.float16)
    cb = np.ascontiguousarray(conv_b.reshape(256, 1))
    w2t = np.ascontiguousarray(
        pcap_w.transpose(1, 2, 3, 0).reshape(2, 128, 81, 256)
    ).astype(np.float16)
    pb = np.ascontiguousarray(pcap_b.reshape(256, 1))

    blkmask = (np.arange(128)[:, None] // 16 ==
               np.arange(128)[None, :] // 16).astype(np.float16)
    ident = np.eye(128, dtype=np.float16)
    repsel = (np.arange(16)[:, None] == (np.arange(128)[None, :] % 16)
              ).astype(np.float16)

    W16 = W.astype(np.float16)
    shared = {"w1t": w1t, "cb": cb, "w2t": w2t, "pb": pb,
              "blkmask": blkmask, "ident": ident, "repsel": repsel}
    in_maps = []
    for c in range(N_CORES):
        n_real = SIZES[c]
        o0 = OFFS[c]
        Wsh = np.zeros((NO, 2048, 16, 8), np.float16)
        Wsh[:n_real] = W16[o0:o0 + n_real]
        # wy[o, oh*16+d, co, k] = W[o, co*8+oh, d, k]
        a = Wsh.reshape(NO, 256, 8, 16, 8)          # [o, co, oh, d, k]
        wy = np.ascontiguousarray(
            a.transpose(0, 2, 3, 1, 4).reshape(NO, 128, 2048)).astype(np.float16)
        # ws[o, p, (h,oh,ow,d)] = W[o, (h*128+p)*8+oh, d, ow]
        b_ = Wsh.reshape(NO, 2, 128, 8, 16, 8)      # [o, h, p, oh, d, ow]
        wsx = b_.transpose(0, 2, 1, 3, 5, 4)        # [o, p, h, oh, ow, d]
        ws = np.ascontiguousarray(wsx.reshape(NO, 128, 2048)).astype(np.float16)
        # ws0[g, p, (j, o*16+d)]: chunk = g*16+j = (h,oh,ow)
        ws0a = wsx.transpose(2, 3, 4, 1, 0, 5).reshape(128, 128, NO * 16)
        ws0 = np.ascontiguousarray(
            ws0a.reshape(8, 16, 128, NO * 16).transpose(0, 2, 1, 3)
            .reshape(8, 128, 16 * NO * 16)).astype(np.float16)
        ob = np.where(np.arange(NO) < n_real, 0.0, -30.0).astype(np.float16)
        obias = np.ascontiguousarray(np.broadcast_to(ob, (128, NO))).copy()

        m = dict(shared)
        m["x_sh"] = np.ascontiguousarray(x[c * B:(c + 1) * B])
        m["wy"] = wy
        m["ws"] = ws
        m["ws0"] = ws0
        m["obias"] = obias.astype(np.float16)
        in_maps.append(m)
    return in_maps


def run(inputs, trace=False, **kw):
    key = "nc"
    if key not in _CACHE:
        _CACHE[key] = _build()
    nc = _CACHE[key]
    in_maps = _host_prep(**inputs)
    res = bass_utils.run_bass_kernel_spmd(
        nc, in_maps, core_ids=list(range(N_CORES)), trace=trace, **kw)
    return res


def kernel(**inputs):
    res = run(inputs)
    v = np.concatenate(
        [res.results[c]["v_out"][:, :SIZES[c], :] for c in range(N_CORES)],
        axis=1)
    return v
